# revision 1
# baseline (speedup 1.0000x reference)
"""Trainium2 Bass kernel for nn_Jitter: out[:, i, :] = x[:, indices[i], :].

Full shapes: x (64, 4096, 256) f32, indices (4096,) int64 -> out (64, 4096, 256) f32.

Strategy: data-parallel over batch dim across 8 NeuronCores (8 batches per
core); the tiny index vector is replicated to every core. On each core the
time-axis gather uses the SWDGE `dma_gather` ucode instruction (production
embedding-gather path): one instruction gathers all 4096 rows (1KB each) of
one batch into a [128, 32, 256] SBUF tile (index n -> partition n%128, chunk
n//128), which an HWDGE DMA then stores to the matching interleaved view of
the output. Memory-bound: each core moves 32MB in + 32MB out.

Indices for dma_gather are int16, wrapped into 16 partitions (idx n ->
partition n%16, col n//16) and replicated to all 128 partitions for the 8
GpSimd cores.
"""

import numpy as np

import concourse.bass as bass
import concourse.tile as tile
from concourse import bacc, mybir
from concourse.bass_utils import run_bass_kernel_spmd
from concourse.library_config import mlp as _mlp_lib

N_CORES = 8
B, T, C = 64, 4096, 256
B_LOC = B // N_CORES  # 8 batches per core
P = 128               # SBUF partitions
J = T // P            # 32 gathered rows per partition
JW = T // 16          # idx tile cols (16-partition wrap)

_CACHE = {}

# The SWDGE descriptor ring holds 1024 descriptors (dynamic_dma_scratch_size
# 16384 / 16B); one dma_gather must stay under that, so split each batch's
# 4096 indices into 4 sub-gathers of 1024.
GSPLIT = 4
IDX_PER_G = T // GSPLIT          # 1024 indices per gather instruction
JW_PER_G = JW // GSPLIT          # 64 idx-tile cols per gather
J_PER_G = J // GSPLIT            # 8 output chunks per gather


def _build(repeat: int = 1, bufs: int = 4):
    """Build + compile the per-core SPMD program.

    repeat: run the whole gather body `repeat` times (for wall-clock delta
            timing in test harnesses); the result is unchanged.
    """
    nc = bacc.Bacc("TRN2", target_bir_lowering=False, debug=False,
                   num_devices=N_CORES)
    x_ext = nc.dram_tensor("x", [B_LOC, T, C], mybir.dt.float32,
                           kind="ExternalInput").ap()
    idx_ext = nc.dram_tensor("idx", [P, JW], mybir.dt.int16,
                             kind="ExternalInput").ap()
    out_ext = nc.dram_tensor("out", [B_LOC, T, C], mybir.dt.float32,
                             kind="ExternalOutput").ap()

    with tile.TileContext(nc) as tc:
        with tc.tile_pool(name="idxp", bufs=1) as idx_pool, \
             tc.tile_pool(name="data", bufs=bufs) as data_pool:
            nc.gpsimd.load_library(_mlp_lib)
            idx_t = idx_pool.tile([P, JW], mybir.dt.int16)
            nc.sync.dma_start(out=idx_t[:], in_=idx_ext[:])
            for _ in range(repeat):
                for b in range(B_LOC):
                    dt = data_pool.tile([P, J, C], mybir.dt.float32)
                    for g in range(GSPLIT):
                        # indices n in [g*1024, (g+1)*1024): local i = n - g*1024
                        # lands at [i % 128, i // 128] of the slice, which is
                        # [n % 128, n // 128] of the full tile (1024 % 128 == 0).
                        nc.gpsimd.dma_gather(
                            dt[:, g * J_PER_G:(g + 1) * J_PER_G, :],
                            x_ext[b],
                            idx_t[:, g * JW_PER_G:(g + 1) * JW_PER_G],
                            num_idxs=IDX_PER_G, num_idxs_reg=IDX_PER_G,
                            elem_size=C,
                        )
                    # gathered index n lives at [n % 128, n // 128, :]
                    out_view = out_ext[b].rearrange("(j p) c -> p j c", p=P)
                    nc.sync.dma_start(out=out_view, in_=dt[:])
    nc.compile()
    return nc


def _prep_idx(indices: np.ndarray) -> np.ndarray:
    idx16 = indices.astype(np.int16)                    # values < 4096 fit
    wrapped = np.ascontiguousarray(idx16.reshape(JW, 16).T)   # [16, JW]
    return np.ascontiguousarray(np.tile(wrapped, (P // 16, 1)))  # [128, JW]


def kernel(x: np.ndarray, indices: np.ndarray) -> np.ndarray:
    key = "main"
    if key not in _CACHE:
        _CACHE[key] = _build()
    nc = _CACHE[key]

    idx_arr = _prep_idx(np.asarray(indices))
    x = np.asarray(x)
    in_maps = [
        {"x": np.ascontiguousarray(x[i * B_LOC:(i + 1) * B_LOC]),
         "idx": idx_arr}
        for i in range(N_CORES)
    ]
    res = run_bass_kernel_spmd(nc, in_maps, list(range(N_CORES)))
    return np.concatenate([res.results[i]["out"] for i in range(N_CORES)],
                          axis=0)



# revision 2
# speedup vs baseline: 12.1560x; 12.1560x over previous
"""Trainium2 Bass kernel for nn_Jitter: out[:, i, :] = x[:, indices[i], :].

Full shapes: x (64, 4096, 256) f32, indices (4096,) int -> out (64, 4096, 256) f32.

Strategy: data-parallel over batch dim across 8 NeuronCores (8 batches per
core); the tiny index vector is replicated to every core. On each core the
time-axis gather uses the SWDGE `dma_gather` ucode instruction (production
embedding-gather path): per batch, 8 gather instructions (512 indices each,
1KB rows) spread round-robin over 4 SWDGE queues pull rows into a
[128, 32, 256] SBUF tile (index n -> partition n%128, chunk n//128), which
an HWDGE DMA (alternating SP/Activation rings) stores to the matching
interleaved view of the output. Memory-bound: each core moves 32MB in +
32MB out; measured ~208us/iter ~= the 64MB/(360GB/s-per-core) DMA roofline.
The 4-queue SWDGE split is what buys the bandwidth - a single queue's
descriptor generation serializes and lands at ~298us.

Indices for dma_gather are int16, wrapped into 16 partitions (idx n ->
partition n%16, col n//16) and replicated to all 128 partitions for the 8
GpSimd cores.
"""

import contextlib

import numpy as np

import concourse.bass as bass
import concourse.tile as tile
from concourse import bacc, mybir
from concourse.bass_utils import run_bass_kernel_spmd
from concourse.library_config import mlp as _mlp_lib

N_CORES = 8
B, T, C = 64, 4096, 256
B_LOC = B // N_CORES  # 8 batches per core
P = 128               # SBUF partitions
J = T // P            # 32 gathered rows per partition
JW = T // 16          # idx tile cols (16-partition wrap)

_CACHE = {}

# Gather granularity: 8 sub-gathers of 512 indices per batch, round-robin
# over 4 SWDGE queues. (The SWDGE descriptor ring holds 1024 descriptors,
# so <=1024 indices per instruction; 512 measured fastest.)
GSPLIT = 8
N_SWDGE_QUEUES = 4
IDX_PER_G = T // GSPLIT          # 512 indices per gather instruction
JW_PER_G = JW // GSPLIT          # 32 idx-tile cols per gather
J_PER_G = J // GSPLIT            # 4 output chunks per gather


def _build(repeat: int = 1, bufs: int = 4):
    """Build + compile the per-core SPMD program.

    repeat: run the whole gather body `repeat` times inside a hardware
            For_i loop (for delta timing in test harnesses); the result
            is unchanged.
    """
    nc = bacc.Bacc("TRN2", target_bir_lowering=False, debug=False,
                   num_devices=N_CORES, num_swdge_queues=N_SWDGE_QUEUES)
    x_ext = nc.dram_tensor("x", [B_LOC, T, C], mybir.dt.float32,
                           kind="ExternalInput").ap()
    idx_ext = nc.dram_tensor("idx", [P, JW], mybir.dt.int16,
                             kind="ExternalInput").ap()
    out_ext = nc.dram_tensor("out", [B_LOC, T, C], mybir.dt.float32,
                             kind="ExternalOutput").ap()

    with tile.TileContext(nc) as tc:
        with tc.tile_pool(name="idxp", bufs=1) as idx_pool, \
             tc.tile_pool(name="data", bufs=bufs) as data_pool:
            nc.gpsimd.load_library(_mlp_lib)
            idx_t = idx_pool.tile([P, JW], mybir.dt.int16)
            nc.sync.dma_start(out=idx_t[:], in_=idx_ext[:])
            loop = tc.For_i(0, repeat) if repeat > 1 else contextlib.nullcontext()
            with loop:
                for b in range(B_LOC):
                    dt = data_pool.tile([P, J, C], mybir.dt.float32)
                    for g in range(GSPLIT):
                        # indices n in [g*512, (g+1)*512): local i = n - g*512
                        # lands at [i % 128, i // 128] of the slice, which is
                        # [n % 128, n // 128] of the full tile (512 % 128 == 0).
                        nc.gpsimd.dma_gather(
                            dt[:, g * J_PER_G:(g + 1) * J_PER_G, :],
                            x_ext[b],
                            idx_t[:, g * JW_PER_G:(g + 1) * JW_PER_G],
                            num_idxs=IDX_PER_G, num_idxs_reg=IDX_PER_G,
                            elem_size=C,
                            queue_num=g % N_SWDGE_QUEUES,
                        )
                    # gathered index n lives at [n % 128, n // 128, :]
                    out_view = out_ext[b].rearrange("(j p) c -> p j c", p=P)
                    eng_s = nc.sync if b % 2 == 0 else nc.scalar
                    eng_s.dma_start(out=out_view, in_=dt[:])
    nc.compile()
    return nc


def _prep_idx(indices: np.ndarray) -> np.ndarray:
    idx16 = indices.astype(np.int16)                    # values < 4096 fit
    wrapped = np.ascontiguousarray(idx16.reshape(JW, 16).T)   # [16, JW]
    return np.ascontiguousarray(np.tile(wrapped, (P // 16, 1)))  # [128, JW]


def kernel(x: np.ndarray, indices: np.ndarray) -> np.ndarray:
    key = "main"
    if key not in _CACHE:
        _CACHE[key] = _build()
    nc = _CACHE[key]

    idx_arr = _prep_idx(np.asarray(indices))
    x = np.asarray(x)
    in_maps = [
        {"x": np.ascontiguousarray(x[i * B_LOC:(i + 1) * B_LOC]),
         "idx": idx_arr}
        for i in range(N_CORES)
    ]
    res = run_bass_kernel_spmd(nc, in_maps, list(range(N_CORES)))
    return np.concatenate([res.results[i]["out"] for i in range(N_CORES)],
                          axis=0)
